# revision 6
# baseline (speedup 1.0000x reference)
"""Multi-head attention (B=2, S=2048, D=1024, H=16, hd=64) on 8 TRN2 NeuronCores.

Sharding: data-parallel over batch (2) x tensor-parallel over heads (4 head-groups
of 4 heads). Core c handles batch c//4, heads [4*(c%4), 4*(c%4)+4).

Per-core program (all matmuls in fp32r = full-rate fp32):
  1. DMA x[b] in, PE-transpose 128x128 blocks -> xT tiles [d_part, seq].
  2. QK projection in [feat, seq] layout (W stationary), V projection in
     [seq, feat] layout (xT stationary). Q pre-scaled by 1/sqrt(hd); q/k bias
     added as per-partition scalars. V bias is folded out: softmax rows sum
     to 1, so attn @ (xWv + b_v) = attn @ xWv + b_v, and the b_v term flows
     through W_o as a constant row added on the host.
  3. Attention per head: scores^T tiles = K^T_tile.T @ Q^T (k on partitions,
     q on free), exp on ACT, then [V | 1]^T @ E accumulates both the
     numerator and the denominator (ones column) in one PSUM group.
  4. Normalize with DVE reciprocal + gpsimd partition_broadcast + DVE mul.
  5. Output projection with values^T stationary; partial outputs summed on
     the host across the 4 head-group cores (plus b_o and the b_v term).
"""

import sys

if "/opt/trn_rl_repo" not in sys.path:
    sys.path.insert(0, "/opt/trn_rl_repo")

import numpy as np

S, D = 2048, 1024
HD = 64
HPC = 4            # heads per core
LF = HPC * HD      # 256 local value feats per core
NQK = 2 * LF       # 512 local q+k feats per core
NCORES = 8
NDT = D // 128     # 8 contraction tiles
NST = S // 128     # 16 seq tiles
NSC = S // 512     # 4 seq chunks
NM = NQK // 128    # 4 qk feat tiles (0,1 = q; 2,3 = k)

_cache = {}


def _build_nc():
    import concourse.tile as tile
    from concourse import bacc, mybir
    from concourse.masks import make_identity
    from contextlib import ExitStack

    F32 = mybir.dt.float32
    F32R = mybir.dt.float32r
    AF = mybir.ActivationFunctionType
    ALU = mybir.AluOpType

    nc = bacc.Bacc("TRN2", target_bir_lowering=False, debug=False)
    x_d = nc.dram_tensor("x", [S, D], F32, kind="ExternalInput").ap()
    wqk_d = nc.dram_tensor("wqk", [D, NQK], F32, kind="ExternalInput").ap()
    bqk_d = nc.dram_tensor("bqk", [NQK, 1], F32, kind="ExternalInput").ap()
    wv_d = nc.dram_tensor("wv", [D, LF], F32, kind="ExternalInput").ap()
    wo_d = nc.dram_tensor("wo", [LF, D], F32, kind="ExternalInput").ap()
    out_d = nc.dram_tensor("out", [S, D], F32, kind="ExternalOutput").ap()

    with ExitStack() as ctx:
        tc = ctx.enter_context(tile.TileContext(nc))
        cpool = ctx.enter_context(tc.tile_pool(name="consts", bufs=1))
        wpool = ctx.enter_context(tc.tile_pool(name="weights", bufs=1))
        wstg_pool = ctx.enter_context(tc.tile_pool(name="wstg", bufs=3))
        xs_pool = ctx.enter_context(tc.tile_pool(name="xs", bufs=6))
        xt_pool = ctx.enter_context(tc.tile_pool(name="xt", bufs=2))
        qkt_pool = ctx.enter_context(tc.tile_pool(name="qkt", bufs=1))
        v1_pool = ctx.enter_context(tc.tile_pool(name="v1", bufs=1))
        e_pool = ctx.enter_context(tc.tile_pool(name="e", bufs=6))
        vt_pool = ctx.enter_context(tc.tile_pool(name="vt", bufs=1))
        sm_pool = ctx.enter_context(tc.tile_pool(name="sm", bufs=2))
        og_pool = ctx.enter_context(tc.tile_pool(name="og", bufs=4))
        psum = ctx.enter_context(tc.tile_pool(name="psum", bufs=2, space="PSUM"))

        ident = cpool.tile([128, 128], F32, name="ident")
        make_identity(nc, ident)

        bqk_t = []
        for m in range(NM):
            t = cpool.tile([128, 1], F32, name=f"bqk{m}", tag=f"bqk{m}")
            nc.sync.dma_start(t[:], bqk_d[m * 128 : (m + 1) * 128, :])
            bqk_t.append(t)

        def load_w(pool, name, tag, src_ap, shape):
            stg = wstg_pool.tile(shape, F32, name=f"{name}_stg", tag="wstg")
            nc.sync.dma_start(stg[:], src_ap)
            t = pool.tile(shape, F32R, name=name, tag=tag)
            nc.gpsimd.tensor_copy(t[:], stg[:])
            return t

        wqkS = [
            load_w(wpool, f"wqkS{d}", f"wqkS{d}", wqk_d[d * 128 : (d + 1) * 128, :], [128, NQK])
            for d in range(NDT)
        ]
        wvS = [
            load_w(wpool, f"wvS{d}", f"wvS{d}", wv_d[d * 128 : (d + 1) * 128, :], [128, LF])
            for d in range(NDT)
        ]
        woS = [
            [
                load_w(
                    wpool,
                    f"woS{fg}_{n}",
                    f"woS{fg}_{n}",
                    wo_d[fg * 128 : (fg + 1) * 128, n * 512 : (n + 1) * 512],
                    [128, 512],
                )
                for n in range(2)
            ]
            for fg in range(2)
        ]

        # [V | 1] tiles: per seq-tile [128 k, 4*(hd+1)]; col 65h+64 holds 1.0
        # (memset can't write f32r, so copy ones in via a strided DVE copy)
        ones4 = cpool.tile([128, HPC], F32, name="ones4")
        nc.vector.memset(ones4[:], 1.0)
        V1 = []
        for i in range(NST):
            t = v1_pool.tile([128, HPC * (HD + 1)], F32R, name=f"V1_{i}", tag=f"V1_{i}")
            nc.vector.tensor_copy(
                t.rearrange("p (h c) -> p h c", c=HD + 1)[:, :, HD : HD + 1],
                ones4[:].unsqueeze(-1),
            )
            V1.append(t)

        qkT = [
            qkt_pool.tile([128, S], F32R, name=f"qkT{m}", tag=f"qkT{m}")
            for m in range(NM)
        ]
        vt = [
            vt_pool.tile([128, S], F32R, name=f"vt{i}", tag=f"vt{i}") for i in range(2)
        ]

        # ---- Phase A: load, transpose, projections (per 512-seq chunk) ----
        for sc in range(NSC):
            xS = []
            for st in range(4):
                t = xs_pool.tile([128, D], F32, name=f"xS_{sc}_{st}", tag="xs")
                nc.sync.dma_start(
                    t[:], x_d[(sc * 4 + st) * 128 : (sc * 4 + st + 1) * 128, :]
                )
                xS.append(t)
            xTc = [
                xt_pool.tile([128, 512], F32R, name=f"xT_{sc}_{d}", tag=f"xt{d}")
                for d in range(NDT)
            ]
            for d in range(NDT):
                pt = psum.tile([128, 512], F32, name=f"tp_{sc}_{d}", tag="tp", bufs=4)
                for st in range(4):
                    nc.tensor.transpose(
                        pt[:, st * 128 : (st + 1) * 128],
                        xS[st][:, d * 128 : (d + 1) * 128],
                        ident[:],
                    )
                nc.vector.tensor_copy(xTc[d][:], pt[:])

            for m in range(NM):
                pq = psum.tile([128, 512], F32, name=f"pq_{sc}_{m}", tag="proj")
                for d in range(NDT):
                    nc.tensor.matmul(
                        pq[:],
                        wqkS[d][:, m * 128 : (m + 1) * 128],
                        xTc[d][:],
                        start=(d == 0),
                        stop=(d == NDT - 1),
                    )
                qscale = 0.125 if m < 2 else 1.0
                nc.vector.tensor_scalar(
                    qkT[m][:, sc * 512 : (sc + 1) * 512],
                    pq[:],
                    qscale,
                    bqk_t[m][:, 0:1],
                    ALU.mult,
                    ALU.add,
                )

            for st in range(4):
                pv = psum.tile([128, LF], F32, name=f"pv_{sc}_{st}", tag="proj")
                for d in range(NDT):
                    nc.tensor.matmul(
                        pv[:],
                        xTc[d][:, st * 128 : (st + 1) * 128],
                        wvS[d][:],
                        start=(d == 0),
                        stop=(d == NDT - 1),
                    )
                it = sc * 4 + st
                for h in range(HPC):
                    nc.scalar.copy(
                        V1[it][:, h * 65 : h * 65 + 64], pv[:, h * 64 : (h + 1) * 64]
                    )

        # ---- Phase B + C: attention per (q-chunk, head), then out-proj ----
        for qc in range(NSC):
            for h in range(HPC):
                qt = qkT[h // 2]
                ktt = qkT[2 + h // 2]
                po = (h % 2) * 64
                pav = psum.tile([128, 512], F32, name=f"av_{qc}_{h}", tag="proj")
                es = []
                for kt in range(NST):
                    ps = psum.tile(
                        [128, 512], F32, name=f"s_{qc}_{h}_{kt}", tag="tp", bufs=4
                    )
                    nc.tensor.matmul(
                        ps[:],
                        ktt[po : po + 64, kt * 128 : (kt + 1) * 128],
                        qt[po : po + 64, qc * 512 : (qc + 1) * 512],
                        start=True,
                        stop=True,
                    )
                    e = e_pool.tile([128, 512], F32R, name=f"e_{qc}_{h}_{kt}", tag="e")
                    nc.scalar.activation(e[:], ps[:], AF.Exp)
                    es.append(e)
                    # issue AV for the previous k-tile AFTER this score matmul
                    # so PE is never stalled waiting on the current exp
                    if kt >= 1:
                        nc.tensor.matmul(
                            pav[0:65, :],
                            V1[kt - 1][:, h * 65 : (h + 1) * 65],
                            es[kt - 1][:],
                            start=(kt == 1),
                            stop=False,
                        )
                nc.tensor.matmul(
                    pav[0:65, :],
                    V1[NST - 1][:, h * 65 : (h + 1) * 65],
                    es[NST - 1][:],
                    start=False,
                    stop=True,
                )
                dr = sm_pool.tile([1, 512], F32, name=f"dr_{qc}_{h}", tag="dr")
                nc.vector.reciprocal(dr[:], pav[64:65, :])
                bc = sm_pool.tile([64, 512], F32, name=f"bc_{qc}_{h}", tag="bc")
                nc.gpsimd.partition_broadcast(bc[:], dr[:])
                nc.vector.tensor_mul(
                    vt[h // 2][po : po + 64, qc * 512 : (qc + 1) * 512],
                    pav[0:64, :],
                    bc[:],
                )

            for mt in range(4):
                mti = qc * 4 + mt
                for n in range(2):
                    pout = psum.tile(
                        [128, 512], F32, name=f"po_{qc}_{mt}_{n}", tag="tp", bufs=4
                    )
                    for fg in range(2):
                        nc.tensor.matmul(
                            pout[:],
                            vt[fg][:, mti * 128 : (mti + 1) * 128],
                            woS[fg][n][:],
                            start=(fg == 0),
                            stop=(fg == 1),
                        )
                    og = og_pool.tile([128, 512], F32, name=f"og_{qc}_{mt}_{n}", tag="og")
                    nc.vector.tensor_copy(og[:], pout[:])
                    nc.sync.dma_start(
                        out_d[mti * 128 : (mti + 1) * 128, n * 512 : (n + 1) * 512],
                        og[:],
                    )

    nc.compile()
    return nc


def get_nc():
    if "nc" not in _cache:
        _cache["nc"] = _build_nc()
    return _cache["nc"]


def make_in_maps(x, W_qkv, b_qkv, W_o):
    # reference packs W_qkv columns per-head: head h owns cols
    # [h*192, h*192+192) as [q_h | k_h | v_h] (64 each)
    in_maps = []
    for c in range(NCORES):
        b, g = divmod(c, 4)
        heads = range(4 * g, 4 * g + 4)
        q = np.concatenate([W_qkv[:, h * 192 : h * 192 + 64] for h in heads], axis=1)
        k = np.concatenate(
            [W_qkv[:, h * 192 + 64 : h * 192 + 128] for h in heads], axis=1
        )
        v = np.concatenate(
            [W_qkv[:, h * 192 + 128 : h * 192 + 192] for h in heads], axis=1
        )
        bq = np.concatenate([b_qkv[h * 192 : h * 192 + 64] for h in heads]) * np.float32(
            0.125
        )
        bk = np.concatenate([b_qkv[h * 192 + 64 : h * 192 + 128] for h in heads])
        lo, hi = g * LF, (g + 1) * LF
        in_maps.append(
            {
                "x": np.ascontiguousarray(x[b], dtype=np.float32),
                "wqk": np.ascontiguousarray(
                    np.concatenate([q, k], axis=1), dtype=np.float32
                ),
                "bqk": np.concatenate([bq, bk]).astype(np.float32).reshape(NQK, 1),
                "wv": np.ascontiguousarray(v, dtype=np.float32),
                "wo": np.ascontiguousarray(W_o[lo:hi, :], dtype=np.float32),
            }
        )
    return in_maps


def kernel(x, W_qkv, b_qkv, W_o, b_o):
    from concourse.bass_utils import run_bass_kernel_spmd

    nc = get_nc()
    in_maps = make_in_maps(x, W_qkv, b_qkv, W_o)
    res = run_bass_kernel_spmd(nc, in_maps, core_ids=list(range(NCORES)))
    out = np.zeros((2, S, D), dtype=np.float32)
    for c in range(NCORES):
        out[c // 4] += res.results[c]["out"]
    b_v = np.concatenate([b_qkv[h * 192 + 128 : h * 192 + 192] for h in range(16)])
    const = (b_v @ W_o + b_o).astype(np.float32)
    out += const[None, None, :]
    return out.astype(np.float32)
